# revision 48
# baseline (speedup 1.0000x reference)
"""Multi-head attention (B=2, S=2048, D=1024, H=16) on 8 Trainium2 cores.

Sharding (Megatron-style): core c handles batch c//4 and heads
[4*(c%4), 4*(c%4)+4). Each core computes its 4 heads' Q/K/V projections,
attention, and a rank-256 partial of the output projection; the host sums
the 4 partials per batch and adds b_o.

Per-core kernel layout:
  - host passes q/k/v transposed ([D, S] bf16) and per-core weight shards
    (w_q/w_k/w_v column shards transposed to [D, 256], w_o row shard
    transposed to [256, D], all bf16)
  - Q, K projected transposed: qT/kT [256, S] (head h at partition
    64*(h%2), chunk h//2) so heads feed the scores matmul directly
  - V projected natural: vp [S, 4, 72-padded] with a constant-1 column
    at index 64 per head; attn@V with lhsT=[v | 1] (m=65) yields the
    softmax denominator in psum row 64 of the same accumulation
  - scoresT = K_chunk^T @ Q with the k=64 contraction zero-padded to 128
    rows so every matmul runs in plain 128x128 mode (64-row-tiled matmuls
    neither register as activity for the PE clock gate nor avoid
    mode-switch drains); the pad costs nothing (matmul time = N columns)
  - exp on ScalarE reads scores psum [128, 1024] directly, emits bf16
  - normalize: DVE reciprocal of the denominator row, partition-broadcast
    via a DRAM-bounce DMA (step-0 source AP), multiply on VectorE
  - output projection from the transposed attention output, evacuated via
    alternating Scalar/Vector copies and DMAed to DRAM
"""
import sys

if "/opt/trn_rl_repo" not in sys.path:
    sys.path.insert(0, "/opt/trn_rl_repo")

import numpy as np
import ml_dtypes

import concourse.bass as bass
import concourse.mybir as mybir
import concourse.tile as tile
from concourse.vector_clock import ScopedClock
from concourse.bass_utils import run_bass_kernel_spmd


BF16 = mybir.dt.bfloat16
F32 = mybir.dt.float32
NPBF16 = ml_dtypes.bfloat16

B, S, D, H = 2, 2048, 1024, 16
DH = 64            # head dim
HPC = 4            # heads per core
DC = HPC * DH      # channels per core (256)
N_CORES = 8
P = 128
KC = D // P        # 8 contraction chunks for projections
SC = S // P        # 16 s-chunks
NT = 512           # matmul free-dim tile
SB = 1024          # attention s-block width (ACT call free dim)

_PROGRAM = None


def _install_drain_split():
    """This walrus build allows only one sync-wait per CTRL instruction;
    spread TileContext's end-of-kernel drain waits over several drains."""

    def _patched(self, tick_clock, wait_clock):
        nc = self.nc
        drain_inst = nc.sync.drain()
        wait_clock.add_sem_waits(
            drain_inst.ins, ScopedClock({None: tick_clock.global_clock})
        )
        waits = list(drain_inst.ins.sync_info.on_wait)
        if len(waits) > 1:
            drain_inst.ins.sync_info.on_wait = waits[:1]
            for w in waits[1:]:
                extra = nc.sync.drain()
                if extra.ins.sync_info is None:
                    extra.ins.sync_info = mybir.SyncInfo(on_wait=[], on_update=[])
                extra.ins.sync_info.on_wait = [w]
        nc.all_engine_barrier()
        assert self.sems is not None
        popped = nc._tile_sem_poison_stack.pop()
        assert popped is self._sem_poison
        nc.clear_and_free_semaphores(list(self.sems.allocated().values()))
        nc.all_engine_barrier()

    tile.TileContext._drain_and_barrier = _patched


def _split_excess_waits(nc):
    """Walrus in this image encodes at most one sync-wait per instruction
    (two for EventSemaphore). Move excess waits onto EventSemaphore
    instructions inserted just before the over-subscribed instruction on
    the same engine."""
    for f in nc.m.functions:
        for bb in f.blocks:
            out_list = []
            for inst in bb.instructions:
                si = inst.sync_info
                waits = list(si.on_wait) if si is not None and si.on_wait else []
                cap = 2 if isinstance(inst, mybir.InstEventSemaphore) else 1
                if len(waits) > cap:
                    si.on_wait = waits[:cap]
                    extra = waits[cap:]
                    for i in range(0, len(extra), 2):
                        ev = mybir.InstEventSemaphore(
                            name=nc.get_next_instruction_name(),
                            opcode="EventSemaphore",
                            engine=inst.engine,
                            ins=[],
                            outs=[],
                            sync_info=mybir.SyncInfo(
                                on_wait=extra[i : i + 2], on_update=[]
                            ),
                            debug=inst.debug,
                        )
                        out_list.append(ev)
                out_list.append(inst)
            bb.instructions[:] = out_list


def _build_program():
    _install_drain_split()
    nc = bass.Bass()

    xqT = nc.declare_dram_parameter("xqT", [D, S], BF16, isOutput=False)
    xkT = nc.declare_dram_parameter("xkT", [D, S], BF16, isOutput=False)
    xvT = nc.declare_dram_parameter("xvT", [D, S], BF16, isOutput=False)
    wqT = nc.declare_dram_parameter("wqT", [D, DC], BF16, isOutput=False)
    wkT = nc.declare_dram_parameter("wkT", [D, DC], BF16, isOutput=False)
    wvT = nc.declare_dram_parameter("wvT", [D, DC], BF16, isOutput=False)
    woT = nc.declare_dram_parameter("woT", [DC, D], BF16, isOutput=False)
    bq = nc.declare_dram_parameter("bq", [DC, 1], F32, isOutput=False)
    bk = nc.declare_dram_parameter("bk", [DC, 1], F32, isOutput=False)
    bv = nc.declare_dram_parameter("bv", [1, DC], BF16, isOutput=False)
    out = nc.declare_dram_parameter("out", [S, D], F32, isOutput=True)

    Exp = mybir.ActivationFunctionType.Exp

    with tile.TileContext(nc) as tc:
        with (
            tc.tile_pool(name="persist", bufs=1) as persist,
            tc.tile_pool(name="work", bufs=1) as work,
        ):
            # ---- persistent sbuf tensors -------------------------------
            qT_sb = persist.tile([P, 2, S], BF16, name="qT_sb")
            kT_sb = persist.tile([P, 2, S], BF16, name="kT_sb")
            # zero-padded K shards per head parity: contraction padded from 64 to
            # 128 rows (zeros) so the scores matmuls run as full 128x128-mode
            # ops — 64-row-tiled matmuls don't register as PE activity for the
            # clock gate and mode switches cost a PE drain. Cost is unchanged
            # (matmul time = moving columns). rhs rows 64-127 multiply zero
            # weights; they only need to be finite.
            kpadE = persist.tile([P, 2, S], BF16, name="kpadE")
            kpadO = persist.tile([P, 2, S], BF16, name="kpadO")
            qoddP = persist.tile([P, 2, S], BF16, name="qoddP")
            vp_sb = persist.tile([P, SC, HPC, 72], BF16, name="vp_sb")
            aoT_sb = persist.tile([P, 2, S], BF16, name="aoT_sb")
            ones_sb = persist.tile([1, P], BF16, name="ones_sb")
            bq_sb = persist.tile([P, 2, 1], F32, name="bq_sb")
            bk_sb = persist.tile([P, 2, 1], F32, name="bk_sb")
            bv_sb = persist.tile([1, DC], BF16, name="bv_sb")
            wo_sb = persist.tile([P, 2, D], BF16, name="wo_sb")

            nc.vector.memset(ones_sb[:, :], 1.0)
            nc.vector.memset(vp_sb[:, :, :, DH : DH + 1], 1.0)
            # small const loads go on the scalar queue (shared with the V
            # inputs, which are needed last) so they don't delay the first
            # projection weight/activation DMAs on sync
            nc.scalar.dma_start(out=bq_sb[:, 0, :], in_=bq[0:P, :])
            nc.scalar.dma_start(out=bq_sb[:, 1, :], in_=bq[P:DC, :])
            nc.scalar.dma_start(out=bk_sb[:, 0, :], in_=bk[0:P, :])
            nc.scalar.dma_start(out=bk_sb[:, 1, :], in_=bk[P:DC, :])
            nc.scalar.dma_start(out=bv_sb[:, :], in_=bv[:, :])
            for m in range(2):
                nc.scalar.dma_start(out=wo_sb[:, m, :], in_=woT[m * P : (m + 1) * P, :])

            # ---- load inputs and weights -------------------------------
            with (
                tc.tile_pool(name="xin", bufs=1) as xin,
                tc.tile_pool(name="psum_proj", bufs=4, space="PSUM") as pp,
            ):
                xq_sb = xin.tile([P, KC, S], BF16, name="xq_sb")
                xk_sb = xin.tile([P, KC, S], BF16, name="xk_sb")
                xv_sb = xin.tile([P, KC, S], BF16, name="xv_sb")
                wq_sb = xin.tile([P, KC, DC], BF16, name="wq_sb")
                wk_sb = xin.tile([P, KC, DC], BF16, name="wk_sb")
                wv_sb = xin.tile([P, KC, DC], BF16, name="wv_sb")
                qs = [nc.sync, nc.gpsimd, nc.scalar]
                for kc in range(KC):
                    r = slice(kc * P, (kc + 1) * P)
                    qs[kc % 3].dma_start(out=wq_sb[:, kc, :], in_=wqT[r, :])
                    qs[(kc + 1) % 3].dma_start(out=wk_sb[:, kc, :], in_=wkT[r, :])
                    qs[(kc + 2) % 3].dma_start(out=wv_sb[:, kc, :], in_=wvT[r, :])
                    qs[kc % 3].dma_start(out=xq_sb[:, kc, :], in_=xqT[r, :])
                    qs[(kc + 1) % 3].dma_start(out=xk_sb[:, kc, :], in_=xkT[r, :])
                    qs[(kc + 2) % 3].dma_start(out=xv_sb[:, kc, :], in_=xvT[r, :])

                # ---- Q/K projections (transposed) ----------------------
                for w_sb, x_sb, o_sb, b_sb in (
                    (wq_sb, xq_sb, qT_sb, bq_sb),
                    (wk_sb, xk_sb, kT_sb, bk_sb),
                ):
                    for m in range(2):
                        for n in range(S // NT):
                            ps = pp.tile([P, NT], F32, name="ps_qk", tag="ps_qk")
                            for kc in range(KC):
                                nc.tensor.matmul(
                                    ps[:, :],
                                    lhsT=w_sb[:, kc, m * P : (m + 1) * P],
                                    rhs=x_sb[:, kc, n * NT : (n + 1) * NT],
                                    start=(kc == 0),
                                    stop=(kc == KC - 1),
                                )
                            nc.vector.tensor_scalar_add(
                                o_sb[:, m, n * NT : (n + 1) * NT],
                                ps[:, :],
                                b_sb[:, m, :],
                            )
                nc.vector.memset(kpadE[DH:P, :, :], 0.0)
                nc.vector.memset(kpadO[DH:P, :, :], 0.0)
                nc.vector.memset(qoddP[DH:P, :, :], 0.0)
                for m in range(2):
                    nc.sync.dma_start(out=kpadE[0:DH, m, :], in_=kT_sb[0:DH, m, :])
                    nc.gpsimd.dma_start(out=kpadO[0:DH, m, :], in_=kT_sb[DH:P, m, :])
                    nc.scalar.dma_start(out=qoddP[0:DH, m, :], in_=qT_sb[DH:P, m, :])

                # ---- V projection (natural layout + bias) --------------
                for mc in range(SC):
                    ps = pp.tile([P, DC], F32, name="ps_v", tag="ps_v")
                    c = slice(mc * P, (mc + 1) * P)
                    for kc in range(KC):
                        nc.tensor.matmul(
                            ps[:, :],
                            lhsT=xv_sb[:, kc, c],
                            rhs=wv_sb[:, kc, :],
                            start=(kc == 0),
                            stop=False,
                        )
                    nc.tensor.matmul(
                        ps[:, :],
                        lhsT=ones_sb[0:1, :],
                        rhs=bv_sb[0:1, :],
                        start=False,
                        stop=True,
                    )
                    nc.vector.tensor_copy(
                        vp_sb[:, mc, :, 0:DH],
                        ps.rearrange("p (h d) -> p h d", h=HPC),
                    )

            # ---- attention: head pairs (0,1) and (2,3) ----------------
            with (
                tc.tile_pool(name="psum_att", bufs=1, space="PSUM") as pa,
                tc.tile_pool(name="ptile", bufs=1) as pt_pool,
                tc.tile_pool(name="dram_scr", bufs=1, space="DRAM") as dscr,
            ):
                for j in range(2):  # head pair = (2j, 2j+1), qT/kT chunk j
                    for sb in range(S // SB):  # s blocks
                        ss = slice(sb * SB, (sb + 1) * SB)
                        o0 = pa.tile([P, SB], F32, name="o0", tag="outT0")
                        o1 = pa.tile([P, SB], F32, name="o1", tag="outT1")
                        for t in range(SC):
                            sc0 = pa.tile([P, SB], F32, name="sc0", tag="scA")
                            sc1 = pa.tile([P, SB], F32, name="sc1", tag="scB")
                            tcol = slice(t * P, (t + 1) * P)
                            for half, sc in ((0, sc0), (1, sc1)):
                                kh = kpadE if half == 0 else kpadO
                                qh = qT_sb if half == 0 else qoddP
                                for nn in range(SB // NT):
                                    nsl = slice(nn * NT, (nn + 1) * NT)
                                    qsl = slice(
                                        sb * SB + nn * NT, sb * SB + (nn + 1) * NT
                                    )
                                    nc.tensor.matmul(
                                        sc[:, nsl],
                                        lhsT=kh[:, j, tcol],
                                        rhs=qh[:, j, qsl],
                                        start=True,
                                        stop=True,
                                    )
                            p0 = pt_pool.tile([P, SB], BF16, name="p0", tag="p0", bufs=2)
                            p1 = pt_pool.tile([P, SB], BF16, name="p1", tag="p1", bufs=2)
                            nc.scalar.activation(p0[:, :], sc0[:, :], Exp, scale=0.125)
                            nc.scalar.activation(p1[:, :], sc1[:, :], Exp, scale=0.125)
                            for o, pt, hh in ((o0, p0, 2 * j), (o1, p1, 2 * j + 1)):
                                for nn in range(SB // NT):
                                    nsl = slice(nn * NT, (nn + 1) * NT)
                                    nc.tensor.matmul(
                                        o[0 : DH + 1, nsl],
                                        lhsT=vp_sb[:, t, hh, 0 : DH + 1],
                                        rhs=pt[:, nsl],
                                        start=(t == 0),
                                        stop=(t == SC - 1),
                                    )
                        # normalize: row DH of o* holds sum(exp(scores)).
                        # Copy psum out to sbuf immediately so the outT psum
                        # tag frees for the next block; the rest of the chain
                        # (reciprocal, DMA broadcast, multiply) runs decoupled.
                        for h, o in ((2 * j, o0), (2 * j + 1, o1)):
                            ocp = work.tile(
                                [DH + 1, SB], F32, name="ocp", tag="ocp", bufs=2
                            )
                            rcf = work.tile([1, SB], F32, name="rcf", tag="rcf", bufs=2)
                            bcs = work.tile([DH, SB], F32, name="bcs", tag="bcs", bufs=2)
                            scr = dscr.tile([1, SB], F32, name="scr", tag="scr", bufs=2)
                            nc.vector.tensor_copy(ocp[:, :], o[0 : DH + 1, :])
                            nc.vector.reciprocal(rcf[:, :], ocp[DH : DH + 1, :])
                            nc.sync.dma_start(out=scr[:, :], in_=rcf[:, :])
                            nc.sync.dma_start(
                                out=bcs[:, :], in_=scr[0:1, :].partition_broadcast(DH)
                            )
                            nc.vector.tensor_mul(
                                aoT_sb[DH * (h % 2) : DH * (h % 2) + DH, j, ss],
                                ocp[0:DH, :],
                                bcs[:, :],
                            )

            # ---- output projection ------------------------------------
            with tc.tile_pool(name="psum_out", bufs=4, space="PSUM") as po:
                for mc in range(SC):
                    c = slice(mc * P, (mc + 1) * P)
                    for n in range(2):
                        nsl = slice(n * NT, (n + 1) * NT)
                        ps = po.tile([P, NT], F32, name="ps_o", tag="ps_o")
                        for kc in range(2):
                            nc.tensor.matmul(
                                ps[:, :],
                                lhsT=aoT_sb[:, kc, c],
                                rhs=wo_sb[:, kc, nsl],
                                start=(kc == 0),
                                stop=(kc == 1),
                            )
                        oo = work.tile([P, NT], F32, name="oo", tag="oo", bufs=4)
                        if (mc + n) % 2:
                            nc.scalar.copy(oo[:, :], ps[:, :])
                        else:
                            nc.vector.tensor_copy(oo[:, :], ps[:, :])
                        (nc.sync if mc % 2 else nc.gpsimd).dma_start(
                            out=out[c, nsl], in_=oo[:, :]
                        )

    _split_excess_waits(nc)
    return nc


def _get_program():
    global _PROGRAM
    if _PROGRAM is None:
        _PROGRAM = _build_program()
    return _PROGRAM


def _shard(inputs):
    q = np.asarray(inputs["q"], np.float32)
    k = np.asarray(inputs["k"], np.float32)
    v = np.asarray(inputs["v"], np.float32)
    w_q = np.asarray(inputs["w_q"], np.float32)
    w_k = np.asarray(inputs["w_k"], np.float32)
    w_v = np.asarray(inputs["w_v"], np.float32)
    w_o = np.asarray(inputs["w_o"], np.float32)
    b_q = np.asarray(inputs["b_q"], np.float32)
    b_k = np.asarray(inputs["b_k"], np.float32)
    b_v = np.asarray(inputs["b_v"], np.float32)

    xT = {}
    for b in range(B):
        xT[("q", b)] = np.ascontiguousarray(q[b].T).astype(NPBF16)
        xT[("k", b)] = np.ascontiguousarray(k[b].T).astype(NPBF16)
        xT[("v", b)] = np.ascontiguousarray(v[b].T).astype(NPBF16)

    in_maps = []
    for c in range(N_CORES):
        b = c // 4
        chs = slice(DC * (c % 4), DC * (c % 4 + 1))
        in_maps.append(
            {
                "xqT": xT[("q", b)],
                "xkT": xT[("k", b)],
                "xvT": xT[("v", b)],
                "wqT": np.ascontiguousarray(w_q[chs].T).astype(NPBF16),
                "wkT": np.ascontiguousarray(w_k[chs].T).astype(NPBF16),
                "wvT": np.ascontiguousarray(w_v[chs].T).astype(NPBF16),
                "woT": np.ascontiguousarray(w_o[:, chs].T).astype(NPBF16),
                "bq": np.ascontiguousarray(b_q[chs].reshape(DC, 1)),
                "bk": np.ascontiguousarray(b_k[chs].reshape(DC, 1)),
                "bv": np.ascontiguousarray(b_v[chs].reshape(1, DC)).astype(NPBF16),
            }
        )
    return in_maps


def kernel(**inputs):
    nc = _get_program()
    in_maps = _shard(inputs)
    res = run_bass_kernel_spmd(nc, in_maps, list(range(N_CORES)))
    b_o = np.asarray(inputs["b_o"], np.float32)
    out = np.zeros((B, S, D), np.float32)
    for c in range(N_CORES):
        out[c // 4] += res.results[c]["out"]
    out += b_o[None, None, :]
    return out


# revision 50
# speedup vs baseline: 1.0615x; 1.0615x over previous
"""Multi-head attention (B=2, S=2048, D=1024, H=16) on 8 Trainium2 cores.

Sharding (Megatron-style): core c handles batch c//4 and heads
[4*(c%4), 4*(c%4)+4). Each core computes its 4 heads' Q/K/V projections,
attention, and a rank-256 partial of the output projection; the host sums
the 4 partials per batch and adds b_o.

Per-core kernel layout:
  - host passes q/k/v transposed ([D, S] bf16) and per-core weight shards
    (w_q/w_k/w_v column shards transposed to [D, 256], w_o row shard
    transposed to [256, D], all bf16)
  - Q, K projected transposed: qT/kT [256, S] (head h at partition
    64*(h%2), chunk h//2) so heads feed the scores matmul directly
  - V projected natural: vp [S, 4, 72-padded] with a constant-1 column
    at index 64 per head; attn@V with lhsT=[v | 1] (m=65) yields the
    softmax denominator in psum row 64 of the same accumulation
  - scoresT = K_chunk^T @ Q with the k=64 contraction zero-padded to 128
    rows so every matmul runs in plain 128x128 mode (64-row-tiled matmuls
    neither register as activity for the PE clock gate nor avoid
    mode-switch drains); the pad costs nothing (matmul time = N columns)
  - exp on ScalarE reads scores psum [128, 1024] directly, emits bf16
  - normalize: DVE reciprocal of the denominator row, partition-broadcast
    via a DRAM-bounce DMA (step-0 source AP), multiply on VectorE
  - output projection from the transposed attention output, evacuated via
    alternating Scalar/Vector copies and DMAed to DRAM
"""
import sys

if "/opt/trn_rl_repo" not in sys.path:
    sys.path.insert(0, "/opt/trn_rl_repo")

import numpy as np
import ml_dtypes

import concourse.bass as bass
import concourse.mybir as mybir
import concourse.tile as tile
from concourse.vector_clock import ScopedClock
from concourse.bass_utils import run_bass_kernel_spmd


BF16 = mybir.dt.bfloat16
F32 = mybir.dt.float32
NPBF16 = ml_dtypes.bfloat16

B, S, D, H = 2, 2048, 1024, 16
DH = 64            # head dim
HPC = 4            # heads per core
DC = HPC * DH      # channels per core (256)
N_CORES = 8
P = 128
KC = D // P        # 8 contraction chunks for projections
SC = S // P        # 16 s-chunks
NT = 512           # matmul free-dim tile
SB = 1024          # attention s-block width (ACT call free dim)

_PROGRAM = None


def _install_drain_split():
    """This walrus build allows only one sync-wait per CTRL instruction;
    spread TileContext's end-of-kernel drain waits over several drains."""

    def _patched(self, tick_clock, wait_clock):
        nc = self.nc
        drain_inst = nc.sync.drain()
        wait_clock.add_sem_waits(
            drain_inst.ins, ScopedClock({None: tick_clock.global_clock})
        )
        waits = list(drain_inst.ins.sync_info.on_wait)
        if len(waits) > 1:
            drain_inst.ins.sync_info.on_wait = waits[:1]
            for w in waits[1:]:
                extra = nc.sync.drain()
                if extra.ins.sync_info is None:
                    extra.ins.sync_info = mybir.SyncInfo(on_wait=[], on_update=[])
                extra.ins.sync_info.on_wait = [w]
        nc.all_engine_barrier()
        assert self.sems is not None
        popped = nc._tile_sem_poison_stack.pop()
        assert popped is self._sem_poison
        nc.clear_and_free_semaphores(list(self.sems.allocated().values()))
        nc.all_engine_barrier()

    tile.TileContext._drain_and_barrier = _patched


def _split_excess_waits(nc):
    """Walrus in this image encodes at most one sync-wait per instruction
    (two for EventSemaphore). Move excess waits onto EventSemaphore
    instructions inserted just before the over-subscribed instruction on
    the same engine."""
    for f in nc.m.functions:
        for bb in f.blocks:
            out_list = []
            for inst in bb.instructions:
                si = inst.sync_info
                waits = list(si.on_wait) if si is not None and si.on_wait else []
                cap = 2 if isinstance(inst, mybir.InstEventSemaphore) else 1
                if len(waits) > cap:
                    si.on_wait = waits[:cap]
                    extra = waits[cap:]
                    for i in range(0, len(extra), 2):
                        ev = mybir.InstEventSemaphore(
                            name=nc.get_next_instruction_name(),
                            opcode="EventSemaphore",
                            engine=inst.engine,
                            ins=[],
                            outs=[],
                            sync_info=mybir.SyncInfo(
                                on_wait=extra[i : i + 2], on_update=[]
                            ),
                            debug=inst.debug,
                        )
                        out_list.append(ev)
                out_list.append(inst)
            bb.instructions[:] = out_list


def _build_program():
    _install_drain_split()
    nc = bass.Bass()

    xqT = nc.declare_dram_parameter("xqT", [D, S], BF16, isOutput=False)
    xkT = nc.declare_dram_parameter("xkT", [D, S], BF16, isOutput=False)
    xvT = nc.declare_dram_parameter("xvT", [D, S], BF16, isOutput=False)
    wqT = nc.declare_dram_parameter("wqT", [D, DC], BF16, isOutput=False)
    wkT = nc.declare_dram_parameter("wkT", [D, DC], BF16, isOutput=False)
    wvT = nc.declare_dram_parameter("wvT", [D, DC], BF16, isOutput=False)
    woT = nc.declare_dram_parameter("woT", [DC, D], BF16, isOutput=False)
    bq = nc.declare_dram_parameter("bq", [DC, 1], F32, isOutput=False)
    bk = nc.declare_dram_parameter("bk", [DC, 1], F32, isOutput=False)
    bv = nc.declare_dram_parameter("bv", [1, DC], F32, isOutput=False)
    out = nc.declare_dram_parameter("out", [S, D], F32, isOutput=True)

    Exp = mybir.ActivationFunctionType.Exp

    with tile.TileContext(nc) as tc:
        with (
            tc.tile_pool(name="persist", bufs=1) as persist,
            tc.tile_pool(name="work", bufs=1) as work,
        ):
            # ---- persistent sbuf tensors -------------------------------
            qT_sb = persist.tile([P, 2, S], BF16, name="qT_sb")
            kT_sb = persist.tile([P, 2, S], BF16, name="kT_sb")
            # zero-padded K shards per head parity: contraction padded from 64 to
            # 128 rows (zeros) so the scores matmuls run as full 128x128-mode
            # ops — 64-row-tiled matmuls don't register as PE activity for the
            # clock gate and mode switches cost a PE drain. Cost is unchanged
            # (matmul time = moving columns). rhs rows 64-127 multiply zero
            # weights; they only need to be finite.
            kpadE = persist.tile([P, 2, S], BF16, name="kpadE")
            kpadO = persist.tile([P, 2, S], BF16, name="kpadO")
            qoddP = persist.tile([P, 2, S], BF16, name="qoddP")
            vp_sb = persist.tile([P, SC, HPC, 72], BF16, name="vp_sb")
            aoT_sb = persist.tile([P, 2, S], BF16, name="aoT_sb")
            ones_sb = persist.tile([1, P], BF16, name="ones_sb")
            bq_sb = persist.tile([P, 2, 1], F32, name="bq_sb")
            bk_sb = persist.tile([P, 2, 1], F32, name="bk_sb")
            bv_sb = persist.tile([1, DC], F32, name="bv_sb")
            bvb_sb = persist.tile([P, DC], F32, name="bvb_sb")
            wo_sb = persist.tile([P, 2, D], BF16, name="wo_sb")

            nc.vector.memset(ones_sb[:, :], 1.0)
            nc.vector.memset(vp_sb[:, :, :, DH : DH + 1], 1.0)
            # small const loads go on the scalar queue (shared with the V
            # inputs, which are needed last) so they don't delay the first
            # projection weight/activation DMAs on sync
            nc.scalar.dma_start(out=bq_sb[:, 0, :], in_=bq[0:P, :])
            nc.scalar.dma_start(out=bq_sb[:, 1, :], in_=bq[P:DC, :])
            nc.scalar.dma_start(out=bk_sb[:, 0, :], in_=bk[0:P, :])
            nc.scalar.dma_start(out=bk_sb[:, 1, :], in_=bk[P:DC, :])
            nc.scalar.dma_start(out=bv_sb[:, :], in_=bv[:, :])
            for m in range(2):
                nc.scalar.dma_start(out=wo_sb[:, m, :], in_=woT[m * P : (m + 1) * P, :])
            nc.scalar.dma_start(out=bvb_sb[:, :], in_=bv[0:1, :].partition_broadcast(P))

            # ---- load inputs and weights -------------------------------
            with (
                tc.tile_pool(name="xin", bufs=1) as xin,
                tc.tile_pool(name="psum_proj", bufs=4, space="PSUM") as pp,
            ):
                xq_sb = xin.tile([P, KC, S], BF16, name="xq_sb")
                xk_sb = xin.tile([P, KC, S], BF16, name="xk_sb")
                xv_sb = xin.tile([P, KC, S], BF16, name="xv_sb")
                wq_sb = xin.tile([P, KC, DC], BF16, name="wq_sb")
                wk_sb = xin.tile([P, KC, DC], BF16, name="wk_sb")
                wv_sb = xin.tile([P, KC, DC], BF16, name="wv_sb")
                qs = [nc.sync, nc.gpsimd, nc.scalar]
                for kc in range(KC):
                    r = slice(kc * P, (kc + 1) * P)
                    qs[kc % 3].dma_start(out=wq_sb[:, kc, :], in_=wqT[r, :])
                    qs[(kc + 1) % 3].dma_start(out=wk_sb[:, kc, :], in_=wkT[r, :])
                    qs[(kc + 2) % 3].dma_start(out=wv_sb[:, kc, :], in_=wvT[r, :])
                    qs[kc % 3].dma_start(out=xq_sb[:, kc, :], in_=xqT[r, :])
                    qs[(kc + 1) % 3].dma_start(out=xk_sb[:, kc, :], in_=xkT[r, :])
                    qs[(kc + 2) % 3].dma_start(out=xv_sb[:, kc, :], in_=xvT[r, :])

                # ---- Q/K projections (transposed) ----------------------
                for w_sb, x_sb, o_sb, b_sb in (
                    (wq_sb, xq_sb, qT_sb, bq_sb),
                    (wk_sb, xk_sb, kT_sb, bk_sb),
                ):
                    for m in range(2):
                        for n in range(S // NT):
                            ps = pp.tile([P, NT], F32, name="ps_qk", tag="ps_qk")
                            for kc in range(KC):
                                nc.tensor.matmul(
                                    ps[:, :],
                                    lhsT=w_sb[:, kc, m * P : (m + 1) * P],
                                    rhs=x_sb[:, kc, n * NT : (n + 1) * NT],
                                    start=(kc == 0),
                                    stop=(kc == KC - 1),
                                )
                            nc.vector.tensor_scalar_add(
                                o_sb[:, m, n * NT : (n + 1) * NT],
                                ps[:, :],
                                b_sb[:, m, :],
                            )
                nc.vector.memset(kpadE[DH:P, :, :], 0.0)
                nc.vector.memset(kpadO[DH:P, :, :], 0.0)
                nc.vector.memset(qoddP[DH:P, :, :], 0.0)
                for m in range(2):
                    nc.sync.dma_start(out=kpadE[0:DH, m, :], in_=kT_sb[0:DH, m, :])
                    nc.gpsimd.dma_start(out=kpadO[0:DH, m, :], in_=kT_sb[DH:P, m, :])
                    nc.scalar.dma_start(out=qoddP[0:DH, m, :], in_=qT_sb[DH:P, m, :])

                # ---- V projection (natural layout + bias) --------------
                for mc in range(SC):
                    ps = pp.tile([P, DC], F32, name="ps_v", tag="ps_v")
                    c = slice(mc * P, (mc + 1) * P)
                    for kc in range(KC):
                        nc.tensor.matmul(
                            ps[:, :],
                            lhsT=xv_sb[:, kc, c],
                            rhs=wv_sb[:, kc, :],
                            start=(kc == 0),
                            stop=(kc == KC - 1),
                        )
                    nc.vector.tensor_add(
                        vp_sb[:, mc, :, 0:DH],
                        ps.rearrange("p (h d) -> p h d", h=HPC),
                        bvb_sb.rearrange("p (h d) -> p h d", h=HPC),
                    )

            # ---- attention: head pairs (0,1) and (2,3) ----------------
            with (
                tc.tile_pool(name="psum_att", bufs=1, space="PSUM") as pa,
                tc.tile_pool(name="ptile", bufs=1) as pt_pool,
                tc.tile_pool(name="dram_scr", bufs=1, space="DRAM") as dscr,
            ):
                for j in range(2):  # head pair = (2j, 2j+1), qT/kT chunk j
                    for sb in range(S // SB):  # s blocks
                        ss = slice(sb * SB, (sb + 1) * SB)
                        o0 = pa.tile([P, SB], F32, name="o0", tag="outT0")
                        o1 = pa.tile([P, SB], F32, name="o1", tag="outT1")
                        for t in range(SC):
                            sc0 = pa.tile([P, SB], F32, name="sc0", tag="scA")
                            sc1 = pa.tile([P, SB], F32, name="sc1", tag="scB")
                            tcol = slice(t * P, (t + 1) * P)
                            for half, sc in ((0, sc0), (1, sc1)):
                                kh = kpadE if half == 0 else kpadO
                                qh = qT_sb if half == 0 else qoddP
                                for nn in range(SB // NT):
                                    nsl = slice(nn * NT, (nn + 1) * NT)
                                    qsl = slice(
                                        sb * SB + nn * NT, sb * SB + (nn + 1) * NT
                                    )
                                    nc.tensor.matmul(
                                        sc[:, nsl],
                                        lhsT=kh[:, j, tcol],
                                        rhs=qh[:, j, qsl],
                                        start=True,
                                        stop=True,
                                    )
                            p0 = pt_pool.tile([P, SB], BF16, name="p0", tag="p0", bufs=2)
                            p1 = pt_pool.tile([P, SB], BF16, name="p1", tag="p1", bufs=2)
                            nc.scalar.activation(p0[:, :], sc0[:, :], Exp, scale=0.125)
                            nc.scalar.activation(p1[:, :], sc1[:, :], Exp, scale=0.125)
                            for o, pt, hh in ((o0, p0, 2 * j), (o1, p1, 2 * j + 1)):
                                for nn in range(SB // NT):
                                    nsl = slice(nn * NT, (nn + 1) * NT)
                                    nc.tensor.matmul(
                                        o[0 : DH + 1, nsl],
                                        lhsT=vp_sb[:, t, hh, 0 : DH + 1],
                                        rhs=pt[:, nsl],
                                        start=(t == 0),
                                        stop=(t == SC - 1),
                                    )
                        # normalize: row DH of o* holds sum(exp(scores)).
                        # Copy psum out to sbuf immediately so the outT psum
                        # tag frees for the next block; the rest of the chain
                        # (reciprocal, DMA broadcast, multiply) runs decoupled.
                        for h, o in ((2 * j, o0), (2 * j + 1, o1)):
                            ocp = work.tile(
                                [DH + 1, SB], F32, name="ocp", tag="ocp", bufs=2
                            )
                            rcf = work.tile([1, SB], F32, name="rcf", tag="rcf", bufs=2)
                            bcs = work.tile([DH, SB], F32, name="bcs", tag="bcs", bufs=2)
                            scr = dscr.tile([1, SB], F32, name="scr", tag="scr", bufs=2)
                            nc.vector.tensor_copy(ocp[:, :], o[0 : DH + 1, :])
                            nc.vector.reciprocal(rcf[:, :], ocp[DH : DH + 1, :])
                            nc.sync.dma_start(out=scr[:, :], in_=rcf[:, :])
                            nc.sync.dma_start(
                                out=bcs[:, :], in_=scr[0:1, :].partition_broadcast(DH)
                            )
                            nc.vector.tensor_mul(
                                aoT_sb[DH * (h % 2) : DH * (h % 2) + DH, j, ss],
                                ocp[0:DH, :],
                                bcs[:, :],
                            )

            # ---- output projection ------------------------------------
            with tc.tile_pool(name="psum_out", bufs=4, space="PSUM") as po:
                for mc in range(SC):
                    c = slice(mc * P, (mc + 1) * P)
                    for n in range(2):
                        nsl = slice(n * NT, (n + 1) * NT)
                        ps = po.tile([P, NT], F32, name="ps_o", tag="ps_o")
                        for kc in range(2):
                            nc.tensor.matmul(
                                ps[:, :],
                                lhsT=aoT_sb[:, kc, c],
                                rhs=wo_sb[:, kc, nsl],
                                start=(kc == 0),
                                stop=(kc == 1),
                            )
                        oo = work.tile([P, NT], F32, name="oo", tag="oo", bufs=4)
                        if (mc + n) % 2:
                            nc.scalar.copy(oo[:, :], ps[:, :])
                        else:
                            nc.vector.tensor_copy(oo[:, :], ps[:, :])
                        qs[(2 * mc + n) % 3].dma_start(out=out[c, nsl], in_=oo[:, :])

    _split_excess_waits(nc)
    return nc


def _get_program():
    global _PROGRAM
    if _PROGRAM is None:
        _PROGRAM = _build_program()
    return _PROGRAM


def _shard(inputs):
    q = np.asarray(inputs["q"], np.float32)
    k = np.asarray(inputs["k"], np.float32)
    v = np.asarray(inputs["v"], np.float32)
    w_q = np.asarray(inputs["w_q"], np.float32)
    w_k = np.asarray(inputs["w_k"], np.float32)
    w_v = np.asarray(inputs["w_v"], np.float32)
    w_o = np.asarray(inputs["w_o"], np.float32)
    b_q = np.asarray(inputs["b_q"], np.float32)
    b_k = np.asarray(inputs["b_k"], np.float32)
    b_v = np.asarray(inputs["b_v"], np.float32)

    xT = {}
    for b in range(B):
        xT[("q", b)] = np.ascontiguousarray(q[b].T).astype(NPBF16)
        xT[("k", b)] = np.ascontiguousarray(k[b].T).astype(NPBF16)
        xT[("v", b)] = np.ascontiguousarray(v[b].T).astype(NPBF16)

    in_maps = []
    for c in range(N_CORES):
        b = c // 4
        chs = slice(DC * (c % 4), DC * (c % 4 + 1))
        in_maps.append(
            {
                "xqT": xT[("q", b)],
                "xkT": xT[("k", b)],
                "xvT": xT[("v", b)],
                "wqT": np.ascontiguousarray(w_q[chs].T).astype(NPBF16),
                "wkT": np.ascontiguousarray(w_k[chs].T).astype(NPBF16),
                "wvT": np.ascontiguousarray(w_v[chs].T).astype(NPBF16),
                "woT": np.ascontiguousarray(w_o[:, chs].T).astype(NPBF16),
                "bq": np.ascontiguousarray(b_q[chs].reshape(DC, 1)),
                "bk": np.ascontiguousarray(b_k[chs].reshape(DC, 1)),
                "bv": np.ascontiguousarray(b_v[chs].reshape(1, DC).astype(np.float32)),
            }
        )
    return in_maps


def kernel(**inputs):
    nc = _get_program()
    in_maps = _shard(inputs)
    res = run_bass_kernel_spmd(nc, in_maps, list(range(N_CORES)))
    b_o = np.asarray(inputs["b_o"], np.float32)
    out = np.zeros((B, S, D), np.float32)
    for c in range(N_CORES):
        out[c // 4] += res.results[c]["out"]
    out += b_o[None, None, :]
    return out
